# revision 30
# baseline (speedup 1.0000x reference)
"""MoE routing kernel for Trainium2 (8 NeuronCores, batch-parallel).

Problem: nn_MoE_47278999994656.
  x [8, 256, 80, 80] f32 + gate Linear(256->5) + 5 experts
  (residual conv1x1 on each 128-ch half, gated by a sigmoid transform),
  top-1 masked-softmax gate => weights are EXACTLY one-hot, so
  out[b] = expert_{argmax_e logits[b,e]}(x[b]).

Sharding: data-parallel over batch, core i computes batch item i.

Design (measured walls: ~230-250 GB/s aggregate DMA per core, Scalar
engine caps the expert pipeline at ~1.7us per 512-px chunk, ~6us fixed
preamble + ~10us fixed framework epilogue inside exec_time):
  - x pre-cast to bf16 on CPU, loaded in 8 chunks of [128, 2, 800] over
    sync/gpsimd/scalar DMA queues; weights ride the same queues.
  - Gate matmuls accumulate on the PE as chunks arrive; junk matmuls
    keep the PE clock ramped through the load + select windows.
  - H weights pre-fused on CPU (Wh = Wt1 @ (I+W)) so H reads x directly.
  - Expert pipeline is software-skewed: D/H/relu for chunk ci issue with
    A/sigmoid/combine for chunk ci-1. D stays in PSUM until the fused
    (D+bias)*sigmoid scalar_tensor_tensor on DVE. Both A matmuls land in
    one 2-bank PSUM tile -> single paired sigmoid on Scalar.
  - GpSimd runs ONLY TensorTensor adds (its TSP ops trigger a ~4us Q7
    library reload that also poisons DVE) + two DMA queue issues.
  - Output staged bf16, DMA'd on sync, upcast on CPU.
"""

import numpy as np

import concourse.bacc as bacc_mod
import concourse.bass as bass
import concourse.mybir as mybir
import concourse.tile as tile
from concourse.bass_utils import run_bass_kernel_spmd

B, C, H, W = 8, 256, 80, 80
HW = H * W          # 6400
HALF = 128
QUARTER = 64
E = 5
NCORES = 8

CHUNKS = [(i * 512, 512) for i in range(12)] + [(6144, 256)]
XCH = [(i * 800, 800) for i in range(8)]       # x DMA chunks
PCH = [(i * 1600, 1600) for i in range(4)]     # gate pooling chunks

UF = 512
F32 = mybir.dt.float32
BF16 = mybir.dt.bfloat16
ALU = mybir.AluOpType
ACT = mybir.ActivationFunctionType


def build_nc() -> bass.Bass:
    nc = bacc_mod.Bacc()

    x_d = nc.dram_tensor("x", [HALF, 2, HW], BF16, kind="ExternalInput")
    u_d = nc.dram_tensor("u", [HALF, E, UF], BF16, kind="ExternalInput")
    bias_d = nc.dram_tensor("bias", [HALF, E, 4], F32, kind="ExternalInput")
    wg_d = nc.dram_tensor("wg", [HALF, 2, E], BF16, kind="ExternalInput")
    bg_d = nc.dram_tensor("bg", [1, E], F32, kind="ExternalInput")
    out_d = nc.dram_tensor("out", [HALF, HW], BF16, kind="ExternalOutput")

    with tile.TileContext(nc) as tc:
        with (
            tc.tile_pool(name="big", bufs=1) as big,
            tc.tile_pool(name="const", bufs=1) as const,
            tc.tile_pool(name="small", bufs=1) as small,
            tc.tile_pool(name="hsb_p", bufs=2) as hsb_p,
            tc.tile_pool(name="ssb_p", bufs=2) as ssb_p,
            tc.tile_pool(name="pp", bufs=4) as pp,
            tc.tile_pool(name="dps", bufs=4, space="PSUM") as dps,
            tc.tile_pool(name="hps", bufs=2, space="PSUM") as hps,
            tc.tile_pool(name="aps", bufs=1, space="PSUM") as aps,
        ):
            # ---- persistent SBUF ----
            xb = big.tile([HALF, 2, HW], BF16)       # 25.6 KB/part
            out_sb = big.tile([HALF, HW], BF16)      # 12.8 KB/part
            u_all = const.tile([HALF, E, UF], BF16)  # 5.1 KB/part
            bias_all = const.tile([HALF, E, 4], F32)
            wg = const.tile([HALF, 2, E], BF16)
            bgx = const.tile([1, E], F32)

            # ---- DMA queue programs (measured-fastest distribution) ----
            # 8 chunks of [128, 2, 800]: sync 0/2/4/6 (+wg/bg), gpsimd
            # 1/3/5 (+u/bias), scalar 7 (after its act-table loads).
            def xdma(q, ci):
                o, n = XCH[ci]
                q.dma_start(out=xb[:, :, o : o + n], in_=x_d[:, :, o : o + n])

            xdma(nc.sync, 0)
            xdma(nc.gpsimd, 1)
            nc.sync.dma_start(out=wg[:], in_=wg_d[:])
            nc.sync.dma_start(out=bgx[:], in_=bg_d[:])
            xdma(nc.sync, 2)
            xdma(nc.gpsimd, 3)
            xdma(nc.sync, 4)
            xdma(nc.gpsimd, 5)
            xdma(nc.sync, 6)
            nc.gpsimd.dma_start(out=u_all[:], in_=u_d[:])
            nc.gpsimd.dma_start(out=bias_all[:], in_=bias_d[:])
            # dummies at scalar front: act-table loads overlap its DMA issues
            ones1 = small.tile([1, HALF], F32)
            nc.vector.memset(ones1, 1.0)
            onesr = small.tile([1, 512], BF16)
            nc.vector.memset(onesr, 1.0)
            onesc = small.tile([1, HALF], BF16)
            nc.vector.memset(onesc, 1.0)
            scr15 = small.tile([1, E], F32)
            scr15b = small.tile([1, E], F32)
            nc.scalar.activation(out=scr15, in_=ones1[0:1, 0:E], func=ACT.Sigmoid)
            nc.scalar.activation(out=scr15b, in_=ones1[0:1, 0:E], func=ACT.Copy)
            xdma(nc.scalar, 7)

            # warm the PE from ~t0 on memset scratch (before x arrives)
            for j in range(8):
                junk0 = hps.tile([HALF, 512], F32, tag="h")
                nc.tensor.matmul(junk0[:, 0:256], lhsT=onesc, rhs=onesr[:, 0:256])

            # ---- gate on PE: yg[5, 512] accumulates per chunk-half ----
            yg = dps.tile([E, 512], F32, tag="d")
            gsl = []
            for o, n in CHUNKS:
                for h in range(2):
                    gsl.append((h, o, n))
            for k, (h, o, n) in enumerate(gsl):
                nc.tensor.matmul(
                    yg[:, 0:n],
                    lhsT=wg[:, h, :],
                    rhs=xb[:, h, o : o + n],
                    start=(k == 0),
                    stop=(k == len(gsl) - 1),
                )

            # junk matmuls keep the PE clock ramped through the select gap
            for j in range(10):
                junk = hps.tile([HALF, 512], F32, tag="h")
                nc.tensor.matmul(
                    junk[:, 0:256], lhsT=xb[:, 0, 0:HALF], rhs=xb[:, 0, 0:256]
                )

            # ---- select: argmax -> one-hot -> weighted weight sum ----
            t32a = small.tile([32, 32], F32)
            t32b = small.tile([32, 32], F32)
            nc.vector.memset(t32a, 0.0)
            nc.vector.reduce_sum(t32a[0:E, 0:1], yg, axis=mybir.AxisListType.X)
            nc.vector.transpose(t32b, t32a)
            lrow = small.tile([1, E], F32)
            nc.vector.tensor_add(lrow, t32b[0:1, 0:E], bgx[0:1, :])
            lmax = small.tile([1, 1], F32)
            nc.vector.reduce_max(lmax, lrow, axis=mybir.AxisListType.X)
            mrow = small.tile([1, E], F32)
            nc.vector.tensor_scalar(
                out=mrow, in0=lrow, scalar1=lmax, scalar2=None,
                op0=ALU.is_equal,
            )
            mps = dps.tile([HALF, E], F32, tag="d")
            nc.tensor.matmul(mps, lhsT=ones1, rhs=mrow)
            mbc = small.tile([HALF, E], F32)
            nc.vector.tensor_copy(mbc, mps)

            # usel halves: D block first (unblocks D matmuls), then H/A.
            # V: m0*, m1*, m4*, chain adds; S: m2*, m3*; G: a23 add.
            usel_d = small.tile([HALF, 256], BF16)
            usel_ha = small.tile([HALF, 256], BF16)
            for usel, base in ((usel_d, 0), (usel_ha, 256)):
                sl = slice(base, base + 256)
                m0 = small.tile([HALF, 256], BF16, tag=f"m0{base}")
                m1 = small.tile([HALF, 256], BF16, tag=f"m1{base}")
                m2 = small.tile([HALF, 256], BF16, tag=f"m2{base}")
                m3 = small.tile([HALF, 256], BF16, tag=f"m3{base}")
                m4 = small.tile([HALF, 256], BF16, tag=f"m4{base}")
                a23 = small.tile([HALF, 256], BF16, tag=f"a23{base}")
                nc.vector.tensor_scalar_mul(m0, u_all[:, 0, sl], mbc[:, 0:1])
                nc.vector.tensor_scalar_mul(m1, u_all[:, 1, sl], mbc[:, 1:2])
                nc.scalar.activation(
                    out=m2, in_=u_all[:, 2, sl], func=ACT.Copy, scale=mbc[:, 2:3]
                )
                nc.scalar.activation(
                    out=m3, in_=u_all[:, 3, sl], func=ACT.Copy, scale=mbc[:, 3:4]
                )
                nc.vector.tensor_scalar_mul(m4, u_all[:, 4, sl], mbc[:, 4:5])
                nc.vector.tensor_add(m0, m0, m1)
                nc.gpsimd.tensor_add(a23, m2, m3)
                nc.vector.tensor_add(m0, m0, m4)
                nc.vector.tensor_add(usel, m0, a23)

            # bsel [128, 4]: cols 0=brgb, 1=btir, 2=bh(stacked), 3=bt2
            bsel = small.tile([HALF, 4], F32)
            nc.scalar.activation(
                out=bsel, in_=bias_all[:, 0, :], func=ACT.Copy, scale=mbc[:, 0:1]
            )
            for e in range(1, E):
                btmp = small.tile([HALF, 4], F32, tag=f"btmp{e}")
                nc.scalar.activation(
                    out=btmp, in_=bias_all[:, e, :], func=ACT.Copy,
                    scale=mbc[:, e : e + 1],
                )
                nc.gpsimd.tensor_add(bsel, bsel, btmp)

            # ---- expert phase: skewed chunk pipeline ----
            NCH = len(CHUNKS)

            def stage_front(ci):
                o, n = CHUNKS[ci]
                dr = dps.tile([HALF, 512], F32, tag="d", name=f"dr{ci}")
                nc.tensor.matmul(
                    dr[:, 0:n], lhsT=usel_d[:, 0:HALF], rhs=xb[:, 0, o : o + n]
                )
                dt = dps.tile([HALF, 512], F32, tag="d", name=f"dt{ci}")
                nc.tensor.matmul(
                    dt[:, 0:n], lhsT=usel_d[:, HALF:256], rhs=xb[:, 1, o : o + n]
                )
                hp = hps.tile([HALF, 512], F32, tag="h", name=f"hp{ci}")
                nc.tensor.matmul(
                    hp[0:QUARTER, 0:n],
                    lhsT=usel_ha[:, 0:QUARTER],
                    rhs=xb[:, 0, o : o + n],
                )
                nc.tensor.matmul(
                    hp[QUARTER:HALF, 0:n],
                    lhsT=usel_ha[:, QUARTER : 2 * QUARTER],
                    rhs=xb[:, 1, o : o + n],
                    tile_position=(0, QUARTER),
                )
                hs = hsb_p.tile([HALF, 512], BF16, tag="hs", name=f"hs{ci}")
                nc.scalar.activation(
                    out=hs[:, 0:n], in_=hp[:, 0:n],
                    func=ACT.Relu, bias=bsel[:, 2:3],
                )
                return dr, dt, hs

            def stage_back(ci, dr, dt, hs):
                o, n = CHUNKS[ci]
                ap2 = aps.tile([HALF, 1024], F32, tag="a", name=f"ap{ci}")
                nc.tensor.matmul(
                    ap2[:, 0:n],
                    lhsT=usel_ha[0:QUARTER, 128:256],
                    rhs=hs[0:QUARTER, 0:n],
                    tile_position=(0, 0),
                )
                nc.tensor.matmul(
                    ap2[:, 512 : 512 + n],
                    lhsT=usel_ha[QUARTER:HALF, 128:256],
                    rhs=hs[QUARTER:HALF, 0:n],
                    tile_position=(QUARTER, 0),
                )
                ss = ssb_p.tile([HALF, 1024], BF16, tag="ss", name=f"ss{ci}")
                if n == 512:
                    nc.scalar.activation(
                        out=ss[:], in_=ap2[:],
                        func=ACT.Sigmoid, bias=bsel[:, 3:4],
                    )
                else:
                    nc.scalar.activation(
                        out=ss[:, 0:n], in_=ap2[:, 0:n],
                        func=ACT.Sigmoid, bias=bsel[:, 3:4],
                    )
                    nc.scalar.activation(
                        out=ss[:, 512 : 512 + n], in_=ap2[:, 512 : 512 + n],
                        func=ACT.Sigmoid, bias=bsel[:, 3:4],
                    )
                p0 = pp.tile([HALF, 512], BF16, tag="p", name=f"p0_{ci}")
                nc.vector.scalar_tensor_tensor(
                    out=p0[:, 0:n], in0=dr[:, 0:n], scalar=bsel[:, 0:1],
                    in1=ss[:, 0:n], op0=ALU.add, op1=ALU.mult,
                )
                p1 = pp.tile([HALF, 512], BF16, tag="p", name=f"p1_{ci}")
                nc.vector.scalar_tensor_tensor(
                    out=p1[:, 0:n], in0=dt[:, 0:n], scalar=bsel[:, 1:2],
                    in1=ss[:, 512 : 512 + n], op0=ALU.add, op1=ALU.mult,
                )
                addeng = nc.vector if ci == NCH - 1 else nc.gpsimd
                addeng.tensor_add(
                    out_sb[:, o : o + n], p0[:, 0:n], p1[:, 0:n]
                )
                # out DMA: pairs, then per-chunk for the tail
                if ci % 2 == 1 and ci < 10:
                    nc.sync.dma_start(
                        out=out_d[:, o - 512 : o + n],
                        in_=out_sb[:, o - 512 : o + n],
                    )
                elif ci >= 10:
                    nc.sync.dma_start(
                        out=out_d[:, o : o + n], in_=out_sb[:, o : o + n]
                    )

            # back(ci-1) emits BEFORE front(ci): the A matmuls reach the
            # PE queue a chunk-slot earlier, so the sigmoid (the Scalar
            # cap chain) isn't left waiting on late A-PSUM writes.
            prev = None
            for ci in range(NCH + 1):
                if prev is not None:
                    stage_back(ci - 1, *prev)
                prev = stage_front(ci) if ci < NCH else None

    nc.compile()
    return nc


def _pack_inputs(x, Wg, bg, Wrgb, brgb, Wtir, btir, Wt1, bt1, Wt2, bt2):
    import ml_dtypes
    eye = np.eye(HALF, dtype=np.float32)
    u = np.zeros((E, HALF, UF), dtype=np.float32)
    bias = np.zeros((E, HALF, 4), dtype=np.float32)
    for e in range(E):
        A0 = eye + Wrgb[e]
        A1 = eye + Wtir[e]
        u[e, :, 0:128] = A0.T
        u[e, :, 128:256] = A1.T
        u[e, :, 256:320] = (Wt1[e] @ A0).T
        u[e, :, 320:384] = (Wt1[e] @ A1).T
        u[e, :, 384:512] = np.tile(
            np.repeat(Wt2[e, 0][:, None], HALF, axis=1), (2, 1)
        )
        bias[e, :, 0] = brgb[e]
        bias[e, :, 1] = btir[e]
        bias[e, 0:QUARTER, 2] = Wt1[e] @ brgb[e] + bt1[e]
        bias[e, QUARTER:HALF, 2] = Wt1[e] @ btir[e] + bt1[e]
        bias[e, :, 3] = bt2[e, 0]
    u = np.ascontiguousarray(u.transpose(1, 0, 2)).astype(ml_dtypes.bfloat16)
    bias = np.ascontiguousarray(bias.transpose(1, 0, 2))

    wgt = Wg.T.astype(np.float32)                   # [256, 5]
    wg_p = np.ascontiguousarray(
        np.stack([wgt[:HALF], wgt[HALF:]], axis=1)
    ).astype(ml_dtypes.bfloat16)
    bgx = np.ascontiguousarray((bg * float(HW))[None, :].astype(np.float32))

    common = {"u": u, "bias": bias, "wg": wg_p, "bg": bgx}
    in_maps = []
    for b in range(B):
        xr = x[b].reshape(2, HALF, HW)              # halves on axis 0
        xp = np.ascontiguousarray(xr.transpose(1, 0, 2)).astype(
            ml_dtypes.bfloat16
        )                                           # [128, 2, 6400]
        m = dict(common)
        m["x"] = xp
        in_maps.append(m)
    return in_maps


_NC_CACHE = {}


def _get_nc():
    if "nc" not in _NC_CACHE:
        _NC_CACHE["nc"] = build_nc()
    return _NC_CACHE["nc"]


def kernel(x, Wg, bg, Wrgb, brgb, Wtir, btir, Wt1, bt1, Wt2, bt2, **run_kw):
    nc = _get_nc()
    in_maps = _pack_inputs(
        np.asarray(x), np.asarray(Wg), np.asarray(bg), np.asarray(Wrgb),
        np.asarray(brgb), np.asarray(Wtir), np.asarray(btir),
        np.asarray(Wt1), np.asarray(bt1), np.asarray(Wt2), np.asarray(bt2),
    )
    res = run_bass_kernel_spmd(nc, in_maps, core_ids=list(range(NCORES)), **run_kw)
    out = np.stack(
        [np.asarray(r["out"]).astype(np.float32) for r in res.results], axis=0
    )                                               # [8, 128, 6400]
    if run_kw:
        kernel.last_results = res
    return out.reshape(B, HALF, H, W)


# revision 31
# speedup vs baseline: 1.0753x; 1.0753x over previous
"""MoE routing kernel for Trainium2 (8 NeuronCores, batch-parallel).

Problem: nn_MoE_47278999994656.
  x [8, 256, 80, 80] f32 + gate Linear(256->5) + 5 experts
  (residual conv1x1 on each 128-ch half, gated by a sigmoid transform),
  top-1 masked-softmax gate => weights are EXACTLY one-hot, so
  out[b] = expert_{argmax_e logits[b,e]}(x[b]).

Sharding: data-parallel over batch, core i computes batch item i.

Design (measured walls: ~230-250 GB/s aggregate DMA per core, Scalar
engine caps the expert pipeline at ~1.7us per 512-px chunk, ~6us fixed
preamble + ~10us fixed framework epilogue inside exec_time):
  - x pre-cast to bf16 on CPU, loaded in 8 chunks of [128, 2, 800] over
    sync/gpsimd/scalar DMA queues; weights ride the same queues.
  - Gate matmuls accumulate on the PE as chunks arrive; junk matmuls
    keep the PE clock ramped through the load + select windows.
  - H weights pre-fused on CPU (Wh = Wt1 @ (I+W)) so H reads x directly.
  - Expert pipeline is software-skewed: D/H/relu for chunk ci issue with
    A/sigmoid/combine for chunk ci-1. D stays in PSUM until the fused
    (D+bias)*sigmoid scalar_tensor_tensor on DVE. Both A matmuls land in
    one 2-bank PSUM tile -> single paired sigmoid on Scalar.
  - GpSimd runs ONLY TensorTensor adds (its TSP ops trigger a ~4us Q7
    library reload that also poisons DVE) + two DMA queue issues.
  - Output staged bf16, DMA'd on sync, upcast on CPU.
"""

import numpy as np

import concourse.bacc as bacc_mod
import concourse.bass as bass
import concourse.mybir as mybir
import concourse.tile as tile
from concourse.bass_utils import run_bass_kernel_spmd

B, C, H, W = 8, 256, 80, 80
HW = H * W          # 6400
HALF = 128
QUARTER = 64
E = 5
NCORES = 8

CHUNKS = [(i * 512, 512) for i in range(12)] + [(6144, 256)]
XCH = [(i * 800, 800) for i in range(8)]       # x DMA chunks
PCH = [(i * 1600, 1600) for i in range(4)]     # gate pooling chunks

UF = 512
F32 = mybir.dt.float32
BF16 = mybir.dt.bfloat16
ALU = mybir.AluOpType
ACT = mybir.ActivationFunctionType


def build_nc() -> bass.Bass:
    nc = bacc_mod.Bacc()

    x_d = nc.dram_tensor("x", [HALF, 2, HW], BF16, kind="ExternalInput")
    u_d = nc.dram_tensor("u", [HALF, E, UF], BF16, kind="ExternalInput")
    bias_d = nc.dram_tensor("bias", [HALF, E, 4], F32, kind="ExternalInput")
    wg_d = nc.dram_tensor("wg", [HALF, 2, E], BF16, kind="ExternalInput")
    bg_d = nc.dram_tensor("bg", [1, E], F32, kind="ExternalInput")
    out_d = nc.dram_tensor("out", [HALF, HW], BF16, kind="ExternalOutput")

    with tile.TileContext(nc) as tc:
        with (
            tc.tile_pool(name="big", bufs=1) as big,
            tc.tile_pool(name="const", bufs=1) as const,
            tc.tile_pool(name="small", bufs=1) as small,
            tc.tile_pool(name="hsb_p", bufs=2) as hsb_p,
            tc.tile_pool(name="ssb_p", bufs=2) as ssb_p,
            tc.tile_pool(name="pp", bufs=4) as pp,
            tc.tile_pool(name="dps", bufs=4, space="PSUM") as dps,
            tc.tile_pool(name="hps", bufs=2, space="PSUM") as hps,
            tc.tile_pool(name="aps", bufs=1, space="PSUM") as aps,
        ):
            # ---- persistent SBUF ----
            xb = big.tile([HALF, 2, HW], BF16)       # 25.6 KB/part
            out_sb = big.tile([HALF, HW], BF16)      # 12.8 KB/part
            u_all = const.tile([HALF, E, UF], BF16)  # 5.1 KB/part
            bias_all = const.tile([HALF, E, 4], F32)
            wg = const.tile([HALF, 2, E], BF16)
            bgx = const.tile([1, E], F32)

            # ---- DMA queue programs (measured-fastest distribution) ----
            # 8 chunks of [128, 2, 800]: sync 0/2/4/6 (+wg/bg), gpsimd
            # 1/3/5 (+u/bias), scalar 7 (after its act-table loads).
            def xdma(q, ci):
                o, n = XCH[ci]
                q.dma_start(out=xb[:, :, o : o + n], in_=x_d[:, :, o : o + n])

            xdma(nc.sync, 0)
            xdma(nc.gpsimd, 1)
            nc.sync.dma_start(out=wg[:], in_=wg_d[:])
            nc.sync.dma_start(out=bgx[:], in_=bg_d[:])
            xdma(nc.sync, 2)
            xdma(nc.gpsimd, 3)
            xdma(nc.sync, 4)
            xdma(nc.gpsimd, 5)
            xdma(nc.sync, 6)
            nc.gpsimd.dma_start(out=u_all[:], in_=u_d[:])
            nc.gpsimd.dma_start(out=bias_all[:], in_=bias_d[:])
            # dummies at scalar front: act-table loads overlap its DMA issues
            ones1 = small.tile([1, HALF], F32)
            nc.vector.memset(ones1, 1.0)
            onesr = small.tile([1, 512], BF16)
            nc.vector.memset(onesr, 1.0)
            onesc = small.tile([1, HALF], BF16)
            nc.vector.memset(onesc, 1.0)
            scr15 = small.tile([1, E], F32)
            scr15b = small.tile([1, E], F32)
            nc.scalar.activation(out=scr15, in_=ones1[0:1, 0:E], func=ACT.Sigmoid)
            nc.scalar.activation(out=scr15b, in_=ones1[0:1, 0:E], func=ACT.Copy)
            xdma(nc.scalar, 7)

            # warm the PE from ~t0 on memset scratch (before x arrives)
            for j in range(8):
                junk0 = hps.tile([HALF, 512], F32, tag="h")
                nc.tensor.matmul(junk0[:, 0:256], lhsT=onesc, rhs=onesr[:, 0:256])

            # ---- gate on PE: yg[5, 512] accumulates per chunk-half ----
            yg = dps.tile([E, 512], F32, tag="d")
            gsl = []
            for o, n in CHUNKS:
                for h in range(2):
                    gsl.append((h, o, n))
            for k, (h, o, n) in enumerate(gsl):
                nc.tensor.matmul(
                    yg[:, 0:n],
                    lhsT=wg[:, h, :],
                    rhs=xb[:, h, o : o + n],
                    start=(k == 0),
                    stop=(k == len(gsl) - 1),
                )

            # junk matmuls keep the PE clock ramped through the select gap
            for j in range(10):
                junk = hps.tile([HALF, 512], F32, tag="h")
                nc.tensor.matmul(
                    junk[:, 0:256], lhsT=xb[:, 0, 0:HALF], rhs=xb[:, 0, 0:256]
                )

            # ---- select: argmax -> one-hot -> weighted weight sum ----
            t32a = small.tile([32, 32], F32)
            t32b = small.tile([32, 32], F32)
            nc.vector.memset(t32a, 0.0)
            nc.vector.reduce_sum(t32a[0:E, 0:1], yg, axis=mybir.AxisListType.X)
            nc.vector.transpose(t32b, t32a)
            lrow = small.tile([1, E], F32)
            nc.vector.tensor_add(lrow, t32b[0:1, 0:E], bgx[0:1, :])
            lmax = small.tile([1, 1], F32)
            nc.vector.reduce_max(lmax, lrow, axis=mybir.AxisListType.X)
            mrow = small.tile([1, E], F32)
            nc.vector.tensor_scalar(
                out=mrow, in0=lrow, scalar1=lmax, scalar2=None,
                op0=ALU.is_equal,
            )
            mps = dps.tile([HALF, E], F32, tag="d")
            nc.tensor.matmul(mps, lhsT=ones1, rhs=mrow)
            mbc = small.tile([HALF, E], F32)
            nc.vector.tensor_copy(mbc, mps)

            # usel halves: D block first (unblocks D matmuls), then H/A.
            # V: m0*, m1*, m4*, chain adds; S: m2*, m3*; G: a23 add.
            usel_d = small.tile([HALF, 256], BF16)
            usel_ha = small.tile([HALF, 256], BF16)
            for usel, base in ((usel_d, 0), (usel_ha, 256)):
                sl = slice(base, base + 256)
                m0 = small.tile([HALF, 256], BF16, tag=f"m0{base}")
                m1 = small.tile([HALF, 256], BF16, tag=f"m1{base}")
                m2 = small.tile([HALF, 256], BF16, tag=f"m2{base}")
                m3 = small.tile([HALF, 256], BF16, tag=f"m3{base}")
                m4 = small.tile([HALF, 256], BF16, tag=f"m4{base}")
                a23 = small.tile([HALF, 256], BF16, tag=f"a23{base}")
                nc.vector.tensor_scalar_mul(m0, u_all[:, 0, sl], mbc[:, 0:1])
                nc.vector.tensor_scalar_mul(m1, u_all[:, 1, sl], mbc[:, 1:2])
                nc.scalar.activation(
                    out=m2, in_=u_all[:, 2, sl], func=ACT.Copy, scale=mbc[:, 2:3]
                )
                nc.scalar.activation(
                    out=m3, in_=u_all[:, 3, sl], func=ACT.Copy, scale=mbc[:, 3:4]
                )
                nc.vector.tensor_scalar_mul(m4, u_all[:, 4, sl], mbc[:, 4:5])
                nc.vector.tensor_add(m0, m0, m1)
                nc.gpsimd.tensor_add(a23, m2, m3)
                nc.vector.tensor_add(m0, m0, m4)
                nc.vector.tensor_add(usel, m0, a23)

            # bsel [128, 4]: cols 0=brgb, 1=btir, 2=bh(stacked), 3=bt2
            bsel = small.tile([HALF, 4], F32)
            nc.scalar.activation(
                out=bsel, in_=bias_all[:, 0, :], func=ACT.Copy, scale=mbc[:, 0:1]
            )
            for e in range(1, E):
                btmp = small.tile([HALF, 4], F32, tag=f"btmp{e}")
                nc.scalar.activation(
                    out=btmp, in_=bias_all[:, e, :], func=ACT.Copy,
                    scale=mbc[:, e : e + 1],
                )
                nc.gpsimd.tensor_add(bsel, bsel, btmp)

            # ---- expert phase: skewed chunk pipeline ----
            NCH = len(CHUNKS)

            def stage_front(ci):
                o, n = CHUNKS[ci]
                dr = dps.tile([HALF, 512], F32, tag="d", name=f"dr{ci}")
                nc.tensor.matmul(
                    dr[:, 0:n], lhsT=usel_d[:, 0:HALF], rhs=xb[:, 0, o : o + n]
                )
                dt = dps.tile([HALF, 512], F32, tag="d", name=f"dt{ci}")
                nc.tensor.matmul(
                    dt[:, 0:n], lhsT=usel_d[:, HALF:256], rhs=xb[:, 1, o : o + n]
                )
                hp = hps.tile([HALF, 512], F32, tag="h", name=f"hp{ci}")
                nc.tensor.matmul(
                    hp[0:QUARTER, 0:n],
                    lhsT=usel_ha[:, 0:QUARTER],
                    rhs=xb[:, 0, o : o + n],
                )
                nc.tensor.matmul(
                    hp[QUARTER:HALF, 0:n],
                    lhsT=usel_ha[:, QUARTER : 2 * QUARTER],
                    rhs=xb[:, 1, o : o + n],
                    tile_position=(0, QUARTER),
                )
                hs = hsb_p.tile([HALF, 512], BF16, tag="hs", name=f"hs{ci}")
                nc.scalar.activation(
                    out=hs[:, 0:n], in_=hp[:, 0:n],
                    func=ACT.Relu, bias=bsel[:, 2:3],
                )
                return dr, dt, hs

            def stage_back(ci, dr, dt, hs):
                o, n = CHUNKS[ci]
                ap2 = aps.tile([HALF, 1024], F32, tag="a", name=f"ap{ci}")
                nc.tensor.matmul(
                    ap2[:, 0:n],
                    lhsT=usel_ha[0:QUARTER, 128:256],
                    rhs=hs[0:QUARTER, 0:n],
                    tile_position=(0, 0),
                )
                nc.tensor.matmul(
                    ap2[:, 512 : 512 + n],
                    lhsT=usel_ha[QUARTER:HALF, 128:256],
                    rhs=hs[QUARTER:HALF, 0:n],
                    tile_position=(QUARTER, 0),
                )
                ss = ssb_p.tile([HALF, 1024], BF16, tag="ss", name=f"ss{ci}")
                if n == 512:
                    nc.scalar.activation(
                        out=ss[:], in_=ap2[:],
                        func=ACT.Sigmoid, bias=bsel[:, 3:4],
                    )
                else:
                    nc.scalar.activation(
                        out=ss[:, 0:n], in_=ap2[:, 0:n],
                        func=ACT.Sigmoid, bias=bsel[:, 3:4],
                    )
                    nc.scalar.activation(
                        out=ss[:, 512 : 512 + n], in_=ap2[:, 512 : 512 + n],
                        func=ACT.Sigmoid, bias=bsel[:, 3:4],
                    )
                p0 = pp.tile([HALF, 512], BF16, tag="p", name=f"p0_{ci}")
                nc.vector.scalar_tensor_tensor(
                    out=p0[:, 0:n], in0=dr[:, 0:n], scalar=bsel[:, 0:1],
                    in1=ss[:, 0:n], op0=ALU.add, op1=ALU.mult,
                )
                p1 = pp.tile([HALF, 512], BF16, tag="p", name=f"p1_{ci}")
                nc.vector.scalar_tensor_tensor(
                    out=p1[:, 0:n], in0=dt[:, 0:n], scalar=bsel[:, 1:2],
                    in1=ss[:, 512 : 512 + n], op0=ALU.add, op1=ALU.mult,
                )
                addeng = nc.vector if ci == NCH - 1 else nc.gpsimd
                addeng.tensor_add(
                    out_sb[:, o : o + n], p0[:, 0:n], p1[:, 0:n]
                )
                # out DMA: pairs, then per-chunk for the tail
                if ci % 2 == 1 and ci < 10:
                    nc.sync.dma_start(
                        out=out_d[:, o - 512 : o + n],
                        in_=out_sb[:, o - 512 : o + n],
                    )
                elif ci >= 10:
                    nc.sync.dma_start(
                        out=out_d[:, o : o + n], in_=out_sb[:, o : o + n]
                    )

            prev = None
            for ci in range(NCH + 1):
                if ci < NCH:
                    cur = stage_front(ci)
                if prev is not None:
                    stage_back(ci - 1, *prev)
                prev = cur if ci < NCH else None

    nc.compile()
    return nc


def _pack_inputs(x, Wg, bg, Wrgb, brgb, Wtir, btir, Wt1, bt1, Wt2, bt2):
    import ml_dtypes
    eye = np.eye(HALF, dtype=np.float32)
    u = np.zeros((E, HALF, UF), dtype=np.float32)
    bias = np.zeros((E, HALF, 4), dtype=np.float32)
    for e in range(E):
        A0 = eye + Wrgb[e]
        A1 = eye + Wtir[e]
        u[e, :, 0:128] = A0.T
        u[e, :, 128:256] = A1.T
        u[e, :, 256:320] = (Wt1[e] @ A0).T
        u[e, :, 320:384] = (Wt1[e] @ A1).T
        u[e, :, 384:512] = np.tile(
            np.repeat(Wt2[e, 0][:, None], HALF, axis=1), (2, 1)
        )
        bias[e, :, 0] = brgb[e]
        bias[e, :, 1] = btir[e]
        bias[e, 0:QUARTER, 2] = Wt1[e] @ brgb[e] + bt1[e]
        bias[e, QUARTER:HALF, 2] = Wt1[e] @ btir[e] + bt1[e]
        bias[e, :, 3] = bt2[e, 0]
    u = np.ascontiguousarray(u.transpose(1, 0, 2)).astype(ml_dtypes.bfloat16)
    bias = np.ascontiguousarray(bias.transpose(1, 0, 2))

    wgt = Wg.T.astype(np.float32)                   # [256, 5]
    wg_p = np.ascontiguousarray(
        np.stack([wgt[:HALF], wgt[HALF:]], axis=1)
    ).astype(ml_dtypes.bfloat16)
    bgx = np.ascontiguousarray((bg * float(HW))[None, :].astype(np.float32))

    common = {"u": u, "bias": bias, "wg": wg_p, "bg": bgx}
    in_maps = []
    for b in range(B):
        xr = x[b].reshape(2, HALF, HW)              # halves on axis 0
        xp = np.ascontiguousarray(xr.transpose(1, 0, 2)).astype(
            ml_dtypes.bfloat16
        )                                           # [128, 2, 6400]
        m = dict(common)
        m["x"] = xp
        in_maps.append(m)
    return in_maps


_NC_CACHE = {}


def _get_nc():
    if "nc" not in _NC_CACHE:
        _NC_CACHE["nc"] = build_nc()
    return _NC_CACHE["nc"]


def kernel(x, Wg, bg, Wrgb, brgb, Wtir, btir, Wt1, bt1, Wt2, bt2, **run_kw):
    nc = _get_nc()
    in_maps = _pack_inputs(
        np.asarray(x), np.asarray(Wg), np.asarray(bg), np.asarray(Wrgb),
        np.asarray(brgb), np.asarray(Wtir), np.asarray(btir),
        np.asarray(Wt1), np.asarray(bt1), np.asarray(Wt2), np.asarray(bt2),
    )
    res = run_bass_kernel_spmd(nc, in_maps, core_ids=list(range(NCORES)), **run_kw)
    out = np.stack(
        [np.asarray(r["out"]).astype(np.float32) for r in res.results], axis=0
    )                                               # [8, 128, 6400]
    if run_kw:
        kernel.last_results = res
    return out.reshape(B, HALF, H, W)
